# revision 3
# baseline (speedup 1.0000x reference)
"""HGRN BitAttention Trainium2 kernel v3 (8-core SPMD, token-sharded).

v3 over v2: Phase X batched (2 passes of 4 token-tiles, rounds on DVE
round-half-even, minimal ACT-table thrash); S/dqb broadcasts via gpsimd
partition_broadcast (no PE involvement); g-phase is pure proj+drain with
the g-norm statistics computed later in the gate phase from the reloaded
gsc chunks; gate DVE chains emitted interleaved into the g-phase and
o-pass emission so the PE queue never stalls on them; stats ones-matmuls
emitted after their inputs are ready.  See v2 docstring for the layout
strategy (all projections feature-major, no transposes, out.T on host).
"""

import numpy as np
import ml_dtypes

import concourse.bass as bass
import concourse.bacc as bacc
import concourse.mybir as mybir
import concourse.tile as tile
from concourse import bass_isa
from concourse.bass_utils import run_bass_kernel_spmd

F32 = mybir.dt.float32
BF16 = mybir.dt.bfloat16
F16 = mybir.dt.float16
I32 = mybir.dt.int32
AF = mybir.ActivationFunctionType
OP = mybir.AluOpType

B, L, D = 4, 2048, 2048
NCORES = 8
TPC = L // 2          # tokens per core = 1024
NTT = TPC // 128      # 8 token tiles per core
KT = D // 128         # 16 k tiles
MT = D // 128         # 16 m tiles
OCH = 4               # gate/o-proj chunks (256 tokens)
OCW = TPC // OCH      # 256
EPS = 1e-5


def build_nc():
    nc = bacc.Bacc("TRN2", target_bir_lowering=False, debug=False,
                   num_devices=NCORES)

    x_d = nc.dram_tensor("x", [TPC, D], F32, kind="ExternalInput")
    wift_d = nc.dram_tensor("wift", [128, MT * 2 * KT * 128], BF16,
                            kind="ExternalInput")   # [p, m, {i,f}, k, c]
    wgt_d = nc.dram_tensor("wgt", [128, MT * KT * 128], BF16,
                           kind="ExternalInput")    # [p, m, k, c]
    wot_d = nc.dram_tensor("wot", [128, MT * KT * 128], BF16,
                           kind="ExternalInput")    # [p, m, k, c]
    gwsc_d = nc.dram_tensor("gwsc", [128, MT], F32, kind="ExternalInput")
    me_d = nc.dram_tensor("mask_even", [128, 1], F32, kind="ExternalInput")
    mo_d = nc.dram_tensor("mask_odd", [128, 1], F32, kind="ExternalInput")
    rws_d = nc.dram_tensor("rws", [128, 4], F32, kind="ExternalInput")
    out_d = nc.dram_tensor("out", [D, TPC], F32, kind="ExternalOutput")

    with tile.TileContext(nc) as tc:
        with (
            tc.tile_pool(name="const", bufs=1) as cp,
            tc.tile_pool(name="hp", bufs=1) as hp,
            tc.tile_pool(name="rows", bufs=1) as rp,
            tc.tile_pool(name="dram", bufs=1, space="DRAM") as dram,
        ):
            # ---- constants ----
            me = cp.tile([128, 1], F32)
            nc.sync.dma_start(me[:], me_d.ap())
            mo = cp.tile([128, 1], F32)
            nc.sync.dma_start(mo[:], mo_d.ap())
            rws = cp.tile([128, 4], F32)
            nc.sync.dma_start(rws[:], rws_d.ap())
            rwsi, rwsf, rwsfn, rwso = (rws[:, i:i + 1] for i in range(4))
            gwsc = cp.tile([128, MT], F32)
            nc.sync.dma_start(gwsc[:], gwsc_d.ap())
            epsb = cp.tile([128, 1], F32)
            nc.vector.memset(epsb[:], EPS)
            ones128 = cp.tile([128, 1], BF16)
            nc.vector.memset(ones128[:], 1.0)

            srec = cp.tile([128, NTT], F32)     # (1/s_x) per token tile col
            bnd = cp.tile([128, MT], F32)
            bnd2 = cp.tile([128, MT], F32)
            carried = cp.tile([128, MT], F32)
            carry_sb = cp.tile([128, MT], F32)

            hsB = hp.tile([128, MT * TPC], F32)           # h, feature-major
            gsc_d = dram.tile([128, MT * TPC], F16)       # g staged via DRAM
            gsc_d3 = gsc_d[:].rearrange("p (m t) -> p m t", m=MT)

            fc_ctx = tc.tile_pool(name="fcp", bufs=1)
            fcp = fc_ctx.__enter__()
            fcB = fcp.tile([128, MT * TPC], F16)          # cumprod(F)

            xq_ctx = tc.tile_pool(name="xqp", bufs=1)
            xqp = xq_ctx.__enter__()
            xqT = xqp.tile([128, KT * TPC], BF16)
            xqT3 = xqT[:].rearrange("p (k t) -> p k t", k=KT)
            S = xqp.tile([128, TPC], F32)       # (1/s_x) broadcast, feat-major

            # =============== Phase X: normalize + quantize x ===============
            with (
                tc.tile_pool(name="xin", bufs=1) as xin,
                tc.tile_pool(name="xw", bufs=2) as xw,
                tc.tile_pool(name="qpl", bufs=2) as qpl,
            ):
                for hf in range(2):
                    xts = []
                    ss4 = xw.tile([128, 4], F32, name="ss4")
                    mx4 = xw.tile([128, 4], F32, name="mx4")
                    for j in range(4):
                        tt = hf * 4 + j
                        xt = xin.tile([128, D], F32, name=f"xt{j}")
                        nc.sync.dma_start(xt[:],
                                          x_d.ap()[tt * 128:(tt + 1) * 128, :])
                        xts.append(xt)
                        scr = qpl.tile([128, D], BF16, name="qscr")
                        nc.scalar.activation(scr[:], xt[:], AF.Square,
                                             accum_out=ss4[:, j:j + 1])
                        nc.vector.tensor_reduce(mx4[:, j:j + 1], xt[:],
                                                mybir.AxisListType.X, OP.max,
                                                apply_absolute_value=True)
                    # batched [128, 4] per-token-tile scale algebra
                    std4 = xw.tile([128, 4], F32, name="std4")
                    nc.scalar.activation(std4[:], ss4[:], AF.Sqrt,
                                         bias=epsb[:], scale=1.0 / D)
                    rstd4 = xw.tile([128, 4], F32, name="rstd4")
                    nc.vector.reciprocal(rstd4[:], std4[:])
                    mxn4 = xw.tile([128, 4], F32, name="mxn4")
                    nc.vector.tensor_tensor(mxn4[:], mx4[:], rstd4[:], OP.mult)
                    nc.vector.tensor_scalar_max(mxn4[:], mxn4[:], EPS)
                    nc.vector.tensor_scalar_mul(srec[:, hf * 4:hf * 4 + 4],
                                                mxn4[:], 1.0 / 127.0)
                    sst4 = xw.tile([128, 4], F32, name="sst4")
                    nc.vector.reciprocal(sst4[:], mxn4[:])
                    rs4 = xw.tile([128, 4], F32, name="rs4")
                    nc.vector.tensor_tensor(rs4[:], rstd4[:], sst4[:], OP.mult)
                    nc.vector.tensor_scalar_mul(rs4[:], rs4[:], 127.0)
                    for j in range(4):
                        tt = hf * 4 + j
                        # DVE f32->i32 cast is round-half-even (= jnp.round)
                        qi = qpl.tile([128, D], I32, name="qi")
                        nc.vector.tensor_scalar_mul(qi[:], xts[j][:],
                                                    rs4[:, j:j + 1])
                        qb = qpl.tile([128, D], BF16, name="qscr")
                        nc.vector.tensor_copy(qb[:], qi[:])
                        nc.sync.dma_start_transpose(
                            xqT3[:, :, tt * 128:(tt + 1) * 128], qb[:])

                # S = broadcast of (1/s) to [128, TPC] feature-major
                srd = dram.tile([1, TPC], F32)
                nc.sync.dma_start(
                    srd[:].rearrange("o (t p) -> (o p) t", p=128), srec[:])
                srow = xw.tile([1, TPC], F32, name="srow")
                nc.sync.dma_start(srow[:], srd[:])
                nc.gpsimd.partition_broadcast(S[:], srow[:])

            # ========= Phase P: i/f projections + scans (feature-major) =========
            with (
                tc.tile_pool(name="wif", bufs=2) as wif,
                tc.tile_pool(name="pw", bufs=2) as pw,
                tc.tile_pool(name="psp", bufs=2, space="PSUM") as psp,
            ):
                for m in range(MT):
                    wm = wif.tile([128, 2 * KT * 128], BF16)
                    nc.sync.dma_start(
                        wm[:], wift_d.ap()[:, m * 2 * KT * 128:
                                           (m + 1) * 2 * KT * 128])
                    psi = psp.tile([128, TPC], F32, name="psi")
                    psf = psp.tile([128, TPC], F32, name="psf")
                    for k in range(KT):
                        wi_k = wm[:, k * 128:(k + 1) * 128]
                        wf_k = wm[:, (KT + k) * 128:(KT + k + 1) * 128]
                        st, sp = (k == 0), (k == KT - 1)
                        nc.tensor.matmul(psi[:, 0:512], wi_k,
                                         xqT[:, k * TPC:k * TPC + 512],
                                         start=st, stop=sp)
                        nc.tensor.matmul(psi[:, 512:TPC], wi_k,
                                         xqT[:, k * TPC + 512:(k + 1) * TPC],
                                         start=st, stop=sp)
                        nc.tensor.matmul(psf[:, 0:512], wf_k,
                                         xqT[:, k * TPC:k * TPC + 512],
                                         start=st, stop=sp)
                        nc.tensor.matmul(psf[:, 512:TPC], wf_k,
                                         xqT[:, k * TPC + 512:(k + 1) * TPC],
                                         start=st, stop=sp)
                    tmpf = pw.tile([128, TPC], F16, name="tmpf")
                    nc.vector.tensor_tensor(tmpf[:], psf[:], S[:], OP.mult)
                    tmpi = pw.tile([128, TPC], F16, name="tmpi")
                    nc.vector.tensor_tensor(tmpi[:], psi[:], S[:], OP.mult)
                    F = pw.tile([128, TPC], F32, name="F")
                    nc.scalar.activation(F[:], tmpf[:], AF.Sigmoid, scale=rwsf)
                    G = pw.tile([128, TPC], F32, name="G")
                    nc.scalar.activation(G[:], tmpf[:], AF.Sigmoid, scale=rwsfn)
                    sil = pw.tile([128, TPC], F32, name="tmpf")
                    nc.scalar.activation(sil[:], tmpi[:], AF.Silu, scale=rwsi)
                    Iin = pw.tile([128, TPC], F32, name="tmpi")
                    nc.vector.tensor_tensor(Iin[:], sil[:], G[:], OP.mult)
                    hm = hsB[:, m * TPC:(m + 1) * TPC]
                    nc.vector.tensor_tensor_scan(hm, F[:], Iin[:], 0.0,
                                                 OP.mult, OP.add)
                    nc.vector.tensor_tensor_scan(
                        fcB[:, m * TPC:(m + 1) * TPC], F[:], F[:], 1.0,
                        OP.mult, OP.bypass)
                    nc.vector.tensor_copy(bnd[:, m:m + 1], hm[:, TPC - 1:TPC])

            # =============== Phase C: carry exchange (async) ===============
            nc.vector.tensor_scalar_mul(bnd2[:], bnd[:], me[:])
            cin = dram.tile([128, MT], F32)
            cout = dram.tile([128, MT], F32)
            nc.sync.dma_start(cin[:], bnd2[:])
            nc.gpsimd.collective_compute(
                "AllReduce", OP.add,
                replica_groups=[[0, 1], [2, 3], [4, 5], [6, 7]],
                ins=[cin.opt()], outs=[cout.opt()],
            )
            nc.sync.dma_start(carry_sb[:], cout[:])
            nc.vector.tensor_scalar_mul(carried[:], carry_sb[:], mo[:])

            def fixup_half(half):
                h0, h1 = half * 512, (half + 1) * 512
                for m in range(MT):
                    hm = hsB[:, m * TPC + h0:m * TPC + h1]
                    nc.vector.scalar_tensor_tensor(
                        hm, fcB[:, m * TPC + h0:m * TPC + h1],
                        carried[:, m:m + 1], hm, OP.mult, OP.add)

            # =============== Phase G: g projection ===============
            hsB3 = hsB[:].rearrange("p (m t) -> p m t", m=MT)
            g_ctx = tc.tile_pool(name="wgp", bufs=2)
            wgp = g_ctx.__enter__()
            gst_ctx = tc.tile_pool(name="gst", bufs=3)
            gst = gst_ctx.__enter__()
            psg_ctx = tc.tile_pool(name="psg", bufs=4, space="PSUM")
            psgp = psg_ctx.__enter__()

            def g_proj(ch):
                cs = ch * 512
                for m in range(MT):
                    wm = wgp.tile([128, KT * 128], BF16, name="wg_m")
                    nc.sync.dma_start(
                        wm[:], wgt_d.ap()[:, m * KT * 128:(m + 1) * KT * 128])
                    psg = psgp.tile([128, 512], F32)
                    for k in range(KT):
                        nc.tensor.matmul(psg[:], wm[:, k * 128:(k + 1) * 128],
                                         xqT[:, k * TPC + cs:k * TPC + cs + 512],
                                         start=(k == 0), stop=(k == KT - 1))
                    gsb = gst.tile([128, 512], F16)
                    nc.vector.scalar_tensor_tensor(
                        gsb[:], psg[:], gwsc[:, m:m + 1], S[:, cs:cs + 512],
                        OP.mult, OP.mult)
                    nc.sync.dma_start(gsc_d3[:, m, cs:cs + 512], gsb[:])

            g_proj(0)
            fixup_half(0)
            fixup_half(1)
            g_proj(1)

            psg_ctx.__exit__(None, None, None)
            gst_ctx.__exit__(None, None, None)
            g_ctx.__exit__(None, None, None)
            xq_ctx.__exit__(None, None, None)
            fc_ctx.__exit__(None, None, None)

            # =============== Phase T ===============
            dq_r = rp.tile([1, TPC], F32)
            gt_ctx = tc.tile_pool(name="gt", bufs=1)
            gt = gt_ctx.__enter__()
            gcl_ctx = tc.tile_pool(name="gcl", bufs=2)
            gcl = gcl_ctx.__enter__()
            dqp_ctx = tc.tile_pool(name="dqp", bufs=4)
            dqp = dqp_ctx.__enter__()
            oq_ctx = tc.tile_pool(name="oq", bufs=4)
            oqp = oq_ctx.__enter__()
            psb_ctx = tc.tile_pool(name="psb", bufs=1, space="PSUM")
            psbp = psb_ctx.__enter__()

            oqcs = [None] * OCH
            dqbs = [None] * OCH
            tsqs = [None] * OCH
            gsqs = [None] * OCH
            mxbs = [None] * OCH

            def gate_dve(ch):
                cs = ch * OCW
                hv = hsB3[:, :, cs:cs + OCW]
                gc = gcl.tile([128, MT * OCW], F16)
                gc3 = gc[:].rearrange("p (m t) -> p m t", m=MT)
                nc.sync.dma_start(gc3[:, :, :], gsc_d3[:, :, cs:cs + OCW])
                hsig = gt.tile([128, MT * OCW], F32, name="hsig")
                hsig3 = hsig[:].rearrange("p (m t) -> p m t", m=MT)
                nc.scalar.activation(hsig3[:, :, :], hv, AF.Silu)
                t_c = gt.tile([128, MT * OCW], F32, name="t_c")
                nc.vector.tensor_tensor(t_c[:], gc[:], hsig[:], OP.mult)
                tsq = gt.tile([128, MT * OCW], BF16, name=f"tsq{ch % 2}")
                nc.scalar.activation(tsq[:], t_c[:], AF.Square)
                gsq = gt.tile([128, MT * OCW], BF16, name=f"gsq{ch % 2}")
                nc.scalar.activation(gsq[:], gc[:], AF.Square)
                tsqs[ch], gsqs[ch] = tsq, gsq
                mxm = gt.tile([128, OCW], F32, name="mxm")
                nc.vector.tensor_reduce(
                    mxm[:], t_c[:].rearrange("p (m t) -> p t m", m=MT),
                    mybir.AxisListType.X, OP.max, apply_absolute_value=True)
                mxb = gt.tile([128, OCW], F32, name=f"mxb{ch % 2}")
                nc.gpsimd.partition_all_reduce(mxb[:], mxm[:], 128,
                                               bass_isa.ReduceOp.max)
                mxbs[ch] = mxb
                csb = gt.tile([128, OCW], F32, name="csb")
                nc.vector.reciprocal(csb[:], mxb[:])
                nc.vector.tensor_scalar_mul(csb[:], csb[:], 127.0)
                uq = gt.tile([128, MT * OCW], I32, name="hsig")
                uq3 = uq[:].rearrange("p (m t) -> p m t", m=MT)
                t3 = t_c[:].rearrange("p (m t) -> p m t", m=MT)
                nc.vector.tensor_tensor(
                    uq3[:, :, :], t3[:, :, :],
                    csb[:].unsqueeze(1).broadcast_to([128, MT, OCW]), OP.mult)
                oqc = oqp.tile([128, MT * OCW], BF16)
                nc.vector.tensor_copy(oqc[:], uq[:])
                oqcs[ch] = oqc

            def stats_rows(ch):
                cs = ch * OCW
                tsq3 = tsqs[ch][:].rearrange("p (m t) -> p m t", m=MT)
                gsq3 = gsqs[ch][:].rearrange("p (m t) -> p m t", m=MT)
                psq = psbp.tile([1, OCW], F32, name="psqo")
                psg2 = psbp.tile([1, OCW], F32, name="psg2")
                for m in range(MT):
                    nc.tensor.matmul(psq[:], ones128[:], tsq3[:, m, :],
                                     start=(m == 0), stop=(m == MT - 1))
                for m in range(MT):
                    nc.tensor.matmul(psg2[:], ones128[:], gsq3[:, m, :],
                                     start=(m == 0), stop=(m == MT - 1))
                stdg = rp.tile([1, OCW], F32, name="stdg")
                nc.scalar.activation(stdg[:], psg2[:], AF.Sqrt,
                                     bias=epsb[0:1, :], scale=1.0 / D)
                rgc = rp.tile([1, OCW], F32, name="rgc")
                nc.vector.reciprocal(rgc[:], stdg[:])
                o2 = rp.tile([1, OCW], F32, name="o2")
                nc.vector.tensor_tensor(o2[:], psq[:], rgc[:], OP.mult)
                nc.vector.tensor_tensor(o2[:], o2[:], rgc[:], OP.mult)
                stdo = rp.tile([1, OCW], F32, name="stdo")
                nc.scalar.activation(stdo[:], o2[:], AF.Sqrt,
                                     bias=epsb[0:1, :], scale=1.0 / D)
                rstdo = rp.tile([1, OCW], F32, name="rstdo")
                nc.vector.reciprocal(rstdo[:], stdo[:])
                mon = rp.tile([1, OCW], F32, name="mon")
                nc.vector.tensor_tensor(mon[:], mxbs[ch][0:1, :], rgc[:],
                                        OP.mult)
                nc.vector.tensor_tensor(mon[:], mon[:], rstdo[:], OP.mult)
                nc.vector.tensor_scalar_max(mon[:], mon[:], EPS)
                nc.vector.tensor_scalar(dq_r[:, cs:cs + OCW], mon[:],
                                        1.0 / 127.0, rws[0:1, 3:4],
                                        OP.mult, OP.mult)
                dqb = dqp.tile([128, OCW], F32)
                nc.gpsimd.partition_broadcast(dqb[:], dq_r[:, cs:cs + OCW])
                dqbs[ch] = dqb

            wos_ctx = tc.tile_pool(name="wos", bufs=2)
            wos = wos_ctx.__enter__()
            ot_ctx = tc.tile_pool(name="ot", bufs=4)
            ot = ot_ctx.__enter__()
            pso_ctx = tc.tile_pool(name="pso", bufs=3, space="PSUM")
            psop = pso_ctx.__enter__()

            opsums = {}

            def o_mms(c0, c1):
                oq0 = oqcs[c0][:].rearrange("p (k t) -> p k t", k=KT)
                oq1 = oqcs[c1][:].rearrange("p (k t) -> p k t", k=KT)
                for m in range(MT):
                    wo_m = wos.tile([128, KT * 128], BF16, name="wo_m")
                    nc.sync.dma_start(
                        wo_m[:],
                        wot_d.ap()[:, m * KT * 128:(m + 1) * KT * 128])
                    pso0 = psop.tile([128, OCW], F32, name="pso0")
                    pso1 = psop.tile([128, OCW], F32, name="pso1")
                    for k in range(KT):
                        wk = wo_m[:, k * 128:(k + 1) * 128]
                        st, sp = (k == 0), (k == KT - 1)
                        nc.tensor.matmul(pso0[:], wk, oq0[:, k, :],
                                         start=st, stop=sp)
                        nc.tensor.matmul(pso1[:], wk, oq1[:, k, :],
                                         start=st, stop=sp)
                    opsums[(c0, m)] = pso0
                    opsums[(c1, m)] = pso1

            def o_drains(c0, c1, ms):
                for m in ms:
                    for ch in (c0, c1):
                        om = ot.tile([128, OCW], F32)
                        nc.vector.tensor_tensor(om[:], opsums[(ch, m)][:],
                                                dqbs[ch][:], OP.mult)
                        nc.sync.dma_start(
                            out_d.ap()[m * 128:(m + 1) * 128,
                                       ch * OCW:(ch + 1) * OCW], om[:])

            # =============== emission schedule ===============
            gate_dve(0)
            gate_dve(1)
            stats_rows(0)
            stats_rows(1)
            o_mms(0, 1)
            o_drains(0, 1, range(0, 6))
            gate_dve(2)
            o_drains(0, 1, range(6, 12))
            gate_dve(3)
            o_drains(0, 1, range(12, 16))
            stats_rows(2)
            stats_rows(3)
            o_mms(2, 3)
            o_drains(2, 3, range(0, 16))

            for ctx in (pso_ctx, ot_ctx, wos_ctx, psb_ctx, oq_ctx, dqp_ctx,
                        gcl_ctx, gt_ctx):
                ctx.__exit__(None, None, None)

    nc.compile()
    return nc


_NC_CACHE = None
LAST_RESULTS = None


def _get_nc():
    global _NC_CACHE
    if _NC_CACHE is None:
        _NC_CACHE = build_nc()
    return _NC_CACHE


def _quant_weight(w):
    import jax
    import jax.numpy as jnp

    mean_abs = np.asarray(
        jax.jit(lambda a: jnp.mean(jnp.abs(a)), backend="cpu")(w)
    )
    ws = np.float32(1.0) / np.maximum(mean_abs.astype(np.float32),
                                      np.float32(1e-5))
    wq = np.clip(np.round(w * ws), -1.0, 1.0).astype(np.float32)
    return wq.T.copy(), np.float32(1.0) / ws


def _tile_fm(wt):
    """WT [d_in, d_out] -> [128p, (m k c)] = WT[k*128+p, m*128+c]."""
    return np.ascontiguousarray(
        wt.reshape(KT, 128, MT, 128).transpose(1, 2, 0, 3).reshape(128, -1)
    ).astype(ml_dtypes.bfloat16)


def kernel(hidden_states, Wi, Wf, Wg, Wo, g_norm_weight):
    nc = _get_nc()

    wiq, rwsi = _quant_weight(np.asarray(Wi))
    wfq, rwsf = _quant_weight(np.asarray(Wf))
    wgq, rwsg = _quant_weight(np.asarray(Wg))
    woq, rwso = _quant_weight(np.asarray(Wo))

    wit = _tile_fm(wiq).reshape(128, MT, KT * 128)
    wft = _tile_fm(wfq).reshape(128, MT, KT * 128)
    wift = np.ascontiguousarray(np.stack([wit, wft], axis=2).reshape(128, -1))
    wgt = _tile_fm(wgq)
    wot = _tile_fm(woq)

    gw = np.asarray(g_norm_weight, dtype=np.float32).reshape(MT, 128)
    gwsc = np.ascontiguousarray(gw.T * rwsg)   # [128, MT]
    x = np.asarray(hidden_states, dtype=np.float32)

    in_maps = []
    for c in range(NCORES):
        b, half = c // 2, c % 2
        rw = np.zeros((128, 4), np.float32)
        rw[:, 0] = rwsi
        rw[:, 1] = rwsf
        rw[:, 2] = -rwsf
        rw[:, 3] = rwso
        in_maps.append({
            "x": np.ascontiguousarray(x[b, half * TPC:(half + 1) * TPC, :]),
            "wift": wift, "wgt": wgt, "wot": wot, "gwsc": gwsc,
            "mask_even": np.full((128, 1), 1.0 - half, np.float32),
            "mask_odd": np.full((128, 1), float(half), np.float32),
            "rws": rw,
        })

    import os
    trace = bool(os.environ.get("HGRN_TRACE"))
    res = run_bass_kernel_spmd(nc, in_maps, list(range(NCORES)), trace=trace)
    global LAST_RESULTS
    LAST_RESULTS = res
    out = np.empty((B, L, D), np.float32)
    for c in range(NCORES):
        b, half = c // 2, c % 2
        out[b, half * TPC:(half + 1) * TPC, :] = res.results[c]["out"].T
    return out
